# revision 1
# baseline (speedup 1.0000x reference)
"""Trainium2 Bass kernel for nn_CodecAttention (sliding-window ALiBi attention).

Reference computation (B=4, T=2048, DIM=1024, H=8, HD=128, WINDOW=16):
    xq = rms_norm(x @ wq) ; xk = rms_norm(x @ wk) ; xv = x @ wv
    scores = q k^T / sqrt(HD) + alibi_bias  (causal + 16-token sliding window)
    out = softmax(scores) @ v  -> reshape -> @ wo

Sharding: 8 cores = (batch b, sequence half). Each core processes 1024 query
tokens plus a 128-token key/value halo (zeros for the first half), fully
locally -- the attention window (16) never crosses the halo, so no
collectives are needed.

Layout strategy (per core): everything transposed. Host passes xT [DIM, 1152].
Projections produce qT/kT in [dim, tok] layout and v in natural [tok, dim]
layout. Scores are computed transposed (sT[k, q] = kT.T @ qT per head), the
softmax denominator comes from a ones-column matmul (reduction over the
partition axis), and PV produces attn_outT[d, q] = v.T-free matmul with
exp(sT) as the moving operand. attn_outT is exactly the stationary operand the
wo matmul wants, so the final output lands in natural [tok, dim] layout with
zero transposes anywhere.

All matmuls run in float32r (full PE rate at N>=256, ~1.6e-4 rel err/K=128).
RMS norm: sum-of-squares via ones-matmul, rsqrt via ACT Sqrt + DVE reciprocal,
applied through a K=1 broadcast matmul (rstd per token broadcast across
partitions; the k-side broadcast uses q_norm_w*k_norm_w/sqrt(HD) as the
stationary operand, folding the norm weights and score scale in for free).

ALiBi + causal + window mask: tiny per-(j) rel tiles with -1e9 at invalid
positions; scores += slope_h * rel via one fused scalar_tensor_tensor. The
first key tile of the first q-chunk additionally subtracts a per-core
"negcol" column that kills out-of-range (global position < 0) halo keys.
"""

import math
import os

import numpy as np

os.environ.setdefault("MYCRO_LOCAL_CACHE", "1")

import concourse.mybir as mybir
import concourse.tile as tile
from concourse import bacc
from concourse.bass_utils import run_bass_kernel_spmd

F32 = mybir.dt.float32
F32R = mybir.dt.float32r
AF = mybir.ActivationFunctionType
ALU = mybir.AluOpType

B, T, DIM = 4, 2048, 1024
H, HD = 8, 128
WINDOW = 16
EPS = 1e-6
NEG = -1.0e9
BIGMASK = 30000.0

HALO = 128                 # key/value halo tokens per shard
TSH = HALO + T // 2        # 1152 tokens per shard
QTOK = T // 2              # 1024 query tokens per shard
ND = DIM // 128            # 8 dim tiles
NT = TSH // 128            # 9 token tiles
QC = 256                   # attention query-chunk width
NQC = QTOK // QC           # 4 query chunks
K_CHUNKS = [(0, 384), (384, 384), (768, 384)]        # kT projection chunks
Q_CHUNKS = [(0, 512), (512, 512)]                    # qT projection chunks

_SLOPES = [2.0 ** (-i) for i in range(H)]

_CACHE = {}


def _build_program():
    nc = bacc.Bacc("TRN2", debug=False, target_bir_lowering=False, num_devices=8)

    xt = nc.declare_dram_parameter("xt", [128, ND, TSH], F32R, isOutput=False)
    wq = nc.declare_dram_parameter("wq", [DIM, DIM], F32R, isOutput=False)
    wk = nc.declare_dram_parameter("wk", [DIM, DIM], F32R, isOutput=False)
    wv = nc.declare_dram_parameter("wv", [DIM, DIM], F32R, isOutput=False)
    wo = nc.declare_dram_parameter("wo", [DIM, DIM], F32R, isOutput=False)
    qkw_row = nc.declare_dram_parameter("qkw_row", [1, ND, 128], F32R, isOutput=False)
    ones_row = nc.declare_dram_parameter("ones_row", [1, 128], F32R, isOutput=False)
    ones_col = nc.declare_dram_parameter("ones_col", [128, 1], F32R, isOutput=False)
    rel4 = nc.declare_dram_parameter("rel4", [128, 4, QC], F32, isOutput=False)
    out = nc.declare_dram_parameter("out", [QTOK, DIM], F32, isOutput=True)

    with tile.TileContext(nc) as tc:
        with tc.tile_pool(name="big", bufs=1) as big:
            # ---- constants + persistent tensors (live for the whole kernel) ----
            kt_sb = big.tile([128, ND, TSH], F32R)
            qt_sb = big.tile([128, ND, QTOK], F32R)
            v_sb = big.tile([128, NT, DIM], F32R)
            qkw_sb = big.tile([1, ND, 128], F32R)
            onesr_sb = big.tile([1, 128], F32R)
            onesc_sb = big.tile([128, 1], F32R)
            rel4_sb = big.tile([128, 4, QC], F32)
            eps_sb = big.tile([1, 1], F32)
            nc.vector.memset(eps_sb[:], EPS)
            nc.sync.dma_start(qkw_sb[:], qkw_row[:])
            nc.sync.dma_start(onesr_sb[:], ones_row[:])
            nc.sync.dma_start(onesc_sb[:], ones_col[:])
            nc.sync.dma_start(rel4_sb[:], rel4[:])

            self_phase1(tc, nc, kt_sb, qt_sb, v_sb, qkw_sb, onesr_sb, onesc_sb,
                        eps_sb, xt, wq, wk, wv)
            self_phase2(tc, nc, kt_sb, qt_sb, v_sb, onesr_sb, onesc_sb,
                        rel4_sb, wo, out)
    nc.compile()
    return nc


def self_phase1(tc, nc, kt_sb, qt_sb, v_sb, qkw_sb, onesr_sb, onesc_sb,
                eps_sb, xt, wq, wk, wv):
    with (
        tc.tile_pool(name="xtp", bufs=1) as xtp,
        tc.tile_pool(name="wp", bufs=int(os.environ.get("KP_WP", 10))) as wp,
        tc.tile_pool(name="scr", bufs=2) as scrp,
        tc.tile_pool(name="sqt", bufs=1) as sqtp,
        tc.tile_pool(name="rcp", bufs=2) as rcpp,
        tc.tile_pool(name="pp", bufs=int(os.environ.get("KP_PP", 6)),
                     space="PSUM") as pp,
        tc.tile_pool(name="sqp", bufs=int(os.environ.get("KP_SQP", 1)),
                     space="PSUM") as sqp,
        tc.tile_pool(name="bcp", bufs=int(os.environ.get("KP_BCP", 1)),
                     space="PSUM") as bcp,
    ):
            xt_sb = xtp.tile([128, ND, TSH], F32R)

            # ---- projections: kT and qT (with RMS-norm), v (plain) ----
            def drain_ps(dst, ps, m, c0, cw, ssq):
                # raw copy (rounded to f32r) + square + ssq accumulation;
                # alternate engines to balance ACT vs DVE load
                if m % 2 == 0:
                    nc.scalar.copy(dst[:, m, c0:c0 + cw], ps[:, :cw])
                else:
                    nc.vector.tensor_copy(dst[:, m, c0:c0 + cw], ps[:, :cw])
                sq = scrp.tile([128, 512], F32R, tag="sq")
                if m % 2 == 0:
                    # DVE square must read the SBUF copy (one-PSUM-input rule)
                    nc.vector.tensor_mul(sq[:, :cw], dst[:, m, c0:c0 + cw],
                                         dst[:, m, c0:c0 + cw])
                else:
                    nc.scalar.square(sq[:, :cw], ps[:, :cw])
                nc.tensor.matmul(
                    ssq[:, :cw], onesc_sb[:], sq[:, :cw],
                    start=(m == 0), stop=(m == ND - 1),
                )

            def proj_normed(w_dram, dst, chunks, tok0, fold_qkw, first=False,
                            pool=None):
                pool = pool or pp
                """dst[:, m, c] = rstd * (x @ w)^T, rstd from raw sum-of-squares."""
                w_slices = []
                for kk in range(ND):
                    w_sl = wp.tile([128, DIM], F32R, tag="wslice")
                    nc.sync.dma_start(w_sl[:], w_dram[kk * 128:(kk + 1) * 128, :])
                    if first:
                        # interleave xt loads so the kk-outer first chunk can
                        # start as soon as the first (w, xt) slice pair lands
                        nc.sync.dma_start(xt_sb[:, kk, :], xt[:, kk, :])
                    w_slices.append(w_sl)
                for ci, (c0, cw) in enumerate(chunks):
                    ssq = sqp.tile([1, 512], F32)
                    if first and ci == 0:
                        # kk-outer in m-blocks of 4: PE consumes DMA'd slices
                        # incrementally instead of waiting for all 16
                        for mb in range(0, ND, 4):
                            blk = []
                            for m in range(mb, mb + 4):
                                ps = pool.tile([128, 512], F32, tag="ps")
                                blk.append(ps)
                            for kk in range(ND):
                                for mi, m in enumerate(range(mb, mb + 4)):
                                    nc.tensor.matmul(
                                        blk[mi][:, :cw],
                                        w_slices[kk][:, m * 128:(m + 1) * 128],
                                        xt_sb[:, kk, tok0 + c0: tok0 + c0 + cw],
                                        start=(kk == 0), stop=(kk == ND - 1),
                                    )
                            for mi, m in enumerate(range(mb, mb + 4)):
                                drain_ps(dst, blk[mi], m, c0, cw, ssq)
                    else:
                        for m in range(ND):
                            ps = pool.tile([128, 512], F32, tag="ps")
                            for kk in range(ND):
                                nc.tensor.matmul(
                                    ps[:, :cw],
                                    w_slices[kk][:, m * 128:(m + 1) * 128],
                                    xt_sb[:, kk, tok0 + c0: tok0 + c0 + cw],
                                    start=(kk == 0), stop=(kk == ND - 1),
                                )
                            drain_ps(dst, ps, m, c0, cw, ssq)
                    sqt = sqtp.tile([1, 512], F32, tag="sqt")
                    nc.scalar.activation(sqt[:, :cw], ssq[:, :cw], AF.Sqrt,
                                         bias=eps_sb[:], scale=1.0 / DIM)
                    rstd = rcpp.tile([1, 512], F32R, tag="rstd")
                    with nc.allow_low_precision(reason="f32r rstd for matmul"):
                        nc.vector.reciprocal(rstd[:, :cw], sqt[:, :cw])
                    if fold_qkw:
                        for m in range(ND):
                            rsb = bcp.tile([128, 512], F32)
                            nc.tensor.matmul(rsb[:, :cw], qkw_sb[:, m, :],
                                             rstd[:, :cw], start=True, stop=True)
                            nc.vector.tensor_mul(dst[:, m, c0:c0 + cw],
                                                 dst[:, m, c0:c0 + cw], rsb[:, :cw])
                    else:
                        rsb = bcp.tile([128, 512], F32)
                        nc.tensor.matmul(rsb[:, :cw], onesr_sb[:],
                                         rstd[:, :cw], start=True, stop=True)
                        # stage the broadcast in SBUF: frees the psum slot and
                        # keeps the 8 muls off the one-PSUM-operand path
                        rsb_sb = scrp.tile([128, 512], F32, tag="rsbsb")
                        nc.scalar.copy(rsb_sb[:, :cw], rsb[:, :cw])
                        for m in range(ND):
                            nc.vector.tensor_mul(dst[:, m, c0:c0 + cw],
                                                 dst[:, m, c0:c0 + cw],
                                                 rsb_sb[:, :cw])

            proj_normed(wk, kt_sb, K_CHUNKS, 0, fold_qkw=True, first=True)
            proj_normed(wq, qt_sb, Q_CHUNKS, HALO, fold_qkw=False)

            # v: natural layout [tok, dim]
            wv_slices = []
            for kk in range(ND):
                w_sl = wp.tile([128, DIM], F32R, tag="wslice")
                nc.sync.dma_start(w_sl[:], wv[kk * 128:(kk + 1) * 128, :])
                wv_slices.append(w_sl)
            for tt in range(NT):
                for nn in range(2):
                    ps = pp.tile([128, 512], F32)
                    for kk in range(ND):
                        nc.tensor.matmul(
                            ps[:],
                            xt_sb[:, kk, tt * 128:(tt + 1) * 128],
                            wv_slices[kk][:, nn * 512:(nn + 1) * 512],
                            start=(kk == 0), stop=(kk == ND - 1),
                        )
                    if tt % 2 == 0:
                        nc.scalar.copy(v_sb[:, tt, nn * 512:(nn + 1) * 512], ps[:])
                    else:
                        nc.vector.tensor_copy(v_sb[:, tt, nn * 512:(nn + 1) * 512],
                                              ps[:])


def self_phase2(tc, nc, kt_sb, qt_sb, v_sb, onesr_sb, onesc_sb,
                rel4_sb, wo, out):
        # xt freed; load wo and run attention + output projection
        with (
            tc.tile_pool(name="wo", bufs=1) as wop,
            tc.tile_pool(name="exp", bufs=int(os.environ.get("KP_EXP", 3))) as expp,
            tc.tile_pool(name="atc", bufs=int(os.environ.get("KP_ATC", 2))) as atcp,
            tc.tile_pool(name="outp", bufs=3) as outp,
            tc.tile_pool(name="rcp2", bufs=2) as rcp2p,
            tc.tile_pool(name="sps", bufs=int(os.environ.get("KP_SPS", 2)),
                         space="PSUM") as sps,
            tc.tile_pool(name="ytp", bufs=int(os.environ.get("KP_YTP", 1)),
                         space="PSUM") as ytp,
            tc.tile_pool(name="rsp", bufs=int(os.environ.get("KP_RSP", 1)),
                         space="PSUM") as rsp,
            tc.tile_pool(name="bc2", bufs=int(os.environ.get("KP_BC2", 1)),
                         space="PSUM") as bc2p,
            tc.tile_pool(name="pso", bufs=int(os.environ.get("KP_PSO", 1)),
                         space="PSUM") as psop,
        ):
            wo_sb = wop.tile([128, ND, DIM], F32R)
            for hd in range(ND):
                nc.sync.dma_start(wo_sb[:, hd, :], wo[hd * 128:(hd + 1) * 128, :])

            for qc in range(NQC):
                aT = atcp.tile([128, ND, QC], F32R)
                for h in range(H):
                    yT_t = ytp.tile([128, QC], F32, tag="yT")
                    rs_t = rsp.tile([1, QC], F32, tag="rs")
                    yT = yT_t[:, :]
                    rs = rs_t[:, :]
                    # joint [128, 3, QC] score tile: three QK matmuls, then ONE
                    # fused bias-add and ONE exp over all 768 columns.
                    # rel4 slots: [0]=j0-first-tile variant (per-core: all-NEG
                    # on first-half cores), [1]=j1, [2]=j2, [3]=j0-regular.
                    # qc=0 uses rel4[0:3] with slots (j0,j1,j2); qc>0 uses
                    # rel4[1:4] with slots (j1,j2,j0).
                    jmap = (0, 1, 2) if qc == 0 else (1, 2, 0)
                    rel_w = rel4_sb[:, 0:3, :] if qc == 0 else rel4_sb[:, 1:4, :]
                    stj = sps.tile([128, 3, QC], F32)
                    st = stj[:, 0:3, :]
                    for s, j in enumerate(jmap):
                        nc.tensor.matmul(
                            stj[:, s, :],
                            kt_sb[:, h, qc * QC + j * 128: qc * QC + (j + 1) * 128],
                            qt_sb[:, h, qc * QC: (qc + 1) * QC],
                            start=True, stop=True,
                        )
                    # scores += slope_h * rel (rel = -1e9 at masked positions)
                    nc.vector.scalar_tensor_tensor(
                        out=st[:], in0=rel_w, scalar=_SLOPES[h],
                        in1=st[:], op0=ALU.mult, op1=ALU.add)
                    ex = expp.tile([128, 3, QC], F32R, tag="exp")
                    nc.scalar.activation(ex[:], st[:], AF.Exp)
                    for s, j in enumerate(jmap):
                        nc.tensor.matmul(
                            yT,
                            v_sb[:, 2 * qc + j, h * 128:(h + 1) * 128],
                            ex[:, s, :], start=(s == 0), stop=(s == 2),
                        )
                        nc.tensor.matmul(
                            rs, onesc_sb[:], ex[:, s, :],
                            start=(s == 0), stop=(s == 2),
                        )
                    rcp = rcp2p.tile([1, QC], F32R, tag="rcp")
                    with nc.allow_low_precision(reason="f32r prob scale"):
                        nc.vector.reciprocal(rcp[:], rs)
                    rsb2_t = bc2p.tile([128, QC], F32, tag="rsb2")
                    rsb2 = rsb2_t[:, :]
                    nc.tensor.matmul(rsb2, onesr_sb[:], rcp[:],
                                     start=True, stop=True)
                    nc.scalar.copy(aT[:, h, :], yT)
                    nc.vector.tensor_mul(aT[:, h, :], aT[:, h, :], rsb2)

                # output projection for this q-chunk
                for t2 in range(QC // 128):
                    for nn in range(2):
                        ps_o = psop.tile([128, 512], F32)
                        for hd in range(ND):
                            nc.tensor.matmul(
                                ps_o[:],
                                aT[:, hd, t2 * 128:(t2 + 1) * 128],
                                wo_sb[:, hd, nn * 512:(nn + 1) * 512],
                                start=(hd == 0), stop=(hd == ND - 1),
                            )
                        o_sb = outp.tile([128, 512], F32, tag="osb")
                        nc.vector.tensor_copy(o_sb[:], ps_o[:])
                        nc.sync.dma_start(
                            out[qc * QC + t2 * 128: qc * QC + (t2 + 1) * 128,
                                nn * 512:(nn + 1) * 512],
                            o_sb[:],
                        )


def _host_constants():
    # relpat(j)[kj, qi] = 128*(j-1) + kj - qi if in window else NEG
    kj = np.arange(128)[:, None, None]
    jj = np.arange(3)[None, :, None]
    qi = np.arange(QC)[None, None, :]
    rel = 128 * (jj - 1) + kj - qi
    valid = (rel <= 0) & (rel >= -WINDOW)
    relpat = np.where(valid, rel, NEG).astype(np.float32)  # [128, 3, QC]
    ones_row = np.ones((1, 128), dtype=np.float32)
    ones_col = np.ones((128, 1), dtype=np.float32)
    return relpat, ones_row, ones_col


def _make_in_maps(x, wq, wk, wv, wo, q_norm_w, k_norm_w):
    x = np.ascontiguousarray(np.asarray(x, dtype=np.float32))
    wq = np.ascontiguousarray(np.asarray(wq, dtype=np.float32))
    wk = np.ascontiguousarray(np.asarray(wk, dtype=np.float32))
    wv = np.ascontiguousarray(np.asarray(wv, dtype=np.float32))
    wo = np.ascontiguousarray(np.asarray(wo, dtype=np.float32))
    q_norm_w = np.asarray(q_norm_w, dtype=np.float32)
    k_norm_w = np.asarray(k_norm_w, dtype=np.float32)

    relpat, ones_row, ones_col = _host_constants()
    qkw = (q_norm_w * k_norm_w / math.sqrt(HD)).astype(np.float32)
    qkw_row = qkw.reshape(1, ND, 128)

    in_maps = []
    for c in range(8):
        b, hf = c // 2, c % 2
        base = hf * (T // 2)
        xsh = np.zeros((TSH, DIM), dtype=np.float32)
        lo = base - HALO
        if lo < 0:
            xsh[HALO:] = x[b, base: base + QTOK]
        else:
            xsh[:] = x[b, lo: base + QTOK]
        xt_c = np.ascontiguousarray(
            xsh.T.reshape(ND, 128, TSH).transpose(1, 0, 2))
        rel4 = np.empty((128, 4, QC), dtype=np.float32)
        rel4[:, 1:3, :] = relpat[:, 1:3, :]          # j1, j2
        rel4[:, 3, :] = relpat[:, 0, :]              # j0 regular
        rel4[:, 0, :] = NEG if hf == 0 else relpat[:, 0, :]  # j0 first tile
        in_maps.append({
            "xt": xt_c, "wq": wq, "wk": wk, "wv": wv, "wo": wo,
            "qkw_row": qkw_row, "ones_row": ones_row, "ones_col": ones_col,
            "rel4": rel4,
        })

    return in_maps


def kernel(x, wq, wk, wv, wo, q_norm_w, k_norm_w):
    if "nc" not in _CACHE:
        _CACHE["nc"] = _build_program()
    nc = _CACHE["nc"]
    in_maps = _make_in_maps(x, wq, wk, wv, wo, q_norm_w, k_norm_w)
    _CACHE["in_maps"] = in_maps
    import time as _time
    last_err = None
    for attempt in range(3):
        try:
            res = run_bass_kernel_spmd(nc, in_maps, core_ids=list(range(8)))
            break
        except Exception as e:  # transient NRT/device wedges recover on retry
            last_err = e
            _time.sleep(10 * (attempt + 1))
    else:
        raise last_err

    out = np.empty((B, T, DIM), dtype=np.float32)
    for c in range(8):
        b, hf = c // 2, c % 2
        out[b, hf * QTOK:(hf + 1) * QTOK, :] = res.results[c]["out"]
    return out



# revision 10
# speedup vs baseline: 1.3767x; 1.3767x over previous
"""Trainium2 Bass kernel for nn_CodecAttention (sliding-window ALiBi attention).

Reference computation (B=4, T=2048, DIM=1024, H=8, HD=128, WINDOW=16):
    xq = rms_norm(x @ wq) ; xk = rms_norm(x @ wk) ; xv = x @ wv
    scores = q k^T / sqrt(HD) + alibi_bias  (causal + 16-token sliding window)
    out = softmax(scores) @ v  -> reshape -> @ wo

Sharding: 8 cores = (batch b, sequence half). Each core processes 1024 query
tokens plus a 128-token key/value halo (zeros for the first half), fully
locally -- the attention window (16) never crosses the halo, so no
collectives are needed.

Speed strategy (vs the f32r baseline):
  * All four big projections run as fp8e4 DoubleRow matmuls (0.5 cycles/row,
    K=256 per instruction). Precision is recovered with a hi/lo residual
    split quantized on the HOST: x = x_hi + x_lo, W*32 = W_hi + W_lo, and
    x@W*32 ~= x_hi@W_hi + x_lo@W_hi + x_hi@W_lo (12 DoubleRow instructions
    per K=1024 output tile = 6N cycles vs f32r's 8N).
  * The norm weights (q_norm*k_norm/sqrt(HD)) are folded into wk on the host;
    RMS-norm stays exact because the sum-of-squares matmul contracts with a
    per-partition 1/(32*qkw)^2 column instead of ones. The 32x fp8 scale
    cancels inside rsqrt for q/k, cancels against the "32" ones-column in the
    softmax normalization for v, and is divided out in the output drain.
  * Attention runs in bf16 (full rate at any width): per 128-query tile only
    2 key tiles are computed (down from 3 per 256), the ALiBi bias + causal +
    window mask are ONE bf16 multiply with a precomputed exp(slope*rel) table
    (zero at masked positions), and the softmax denominator comes free as a
    129th "32.0" column appended to V (the 32 cancels V's fp8 scale).
    Normalization is a per-partition ACT copy with scale=1/Z, then a PE
    transpose puts the result back into [head_dim, token] layout for wo.
"""

import math
import os

import numpy as np
import ml_dtypes

os.environ.setdefault("MYCRO_LOCAL_CACHE", "1")

import concourse.mybir as mybir
import concourse.tile as tile
from concourse import bacc
from concourse.bass_utils import run_bass_kernel_spmd

F32 = mybir.dt.float32
BF16 = mybir.dt.bfloat16
F8 = mybir.dt.float8e4
DR = mybir.MatmulPerfMode.DoubleRow
AF = mybir.ActivationFunctionType
ALU = mybir.AluOpType
E4NP = ml_dtypes.float8_e4m3
BFNP = ml_dtypes.bfloat16

B, T, DIM = 4, 2048, 1024
H, HD = 8, 128
WINDOW = 16
EPS = 1e-6
WS = 32.0                  # fp8 weight scale
VCOL = WS                  # value of the ones-column appended to V

HALO = 128                 # key/value halo tokens per shard
TSH = HALO + T // 2        # 1152 tokens per shard
QTOK = T // 2              # 1024 query tokens per shard
NP4 = 4                    # K=1024 -> 4 DoubleRow pairs
NT = TSH // 128            # 9 token tiles
NQT = QTOK // 128          # 8 query tiles
K_CHUNKS = [(0, 384), (384, 384), (768, 384)]        # kT projection chunks
Q_CHUNKS = [(0, 512), (512, 512)]                    # qT projection chunks

_SLOPES = [2.0 ** (-i) for i in range(H)]

_CACHE = {}


def _build_program():
    nc = bacc.Bacc("TRN2", debug=False, target_bir_lowering=False, num_devices=8)

    xt_hi = nc.declare_dram_parameter("xt_hi", [128, NP4, 2, TSH], F8, isOutput=False)
    xt_lo = nc.declare_dram_parameter("xt_lo", [128, NP4, 2, TSH], F8, isOutput=False)
    wps = {}
    for nm in ("wk", "wq", "wv", "wo"):
        for half in ("hi", "lo"):
            wps[f"{nm}_{half}"] = nc.declare_dram_parameter(
                f"{nm}_{half}", [128, NP4, 2, DIM], F8, isOutput=False)
    xtab = nc.declare_dram_parameter("xtab", [128, H, 4, 128], BF16, isOutput=False)
    ident = nc.declare_dram_parameter("ident", [128, 128], BF16, isOutput=False)
    invq2 = nc.declare_dram_parameter("invq2", [128, 2, 8], BF16, isOutput=False)
    ones_row = nc.declare_dram_parameter("ones_row", [1, 128], BF16, isOutput=False)
    out = nc.declare_dram_parameter("out", [QTOK, DIM], F32, isOutput=True)

    with tile.TileContext(nc) as tc:
        with tc.tile_pool(name="big", bufs=1) as big:
            # ---- persistent SBUF tensors ----
            xhi_sb = big.tile([128, NP4, 2, TSH], F8)
            xlo_sb = big.tile([128, NP4, 2, TSH], F8)
            w_sb = {k: big.tile([128, NP4, 2, DIM], F8, name=f"w_{k}")
                    for k in wps}
            kt_sb = big.tile([128, H, TSH], BF16)
            qt_sb = big.tile([128, H, QTOK], BF16)
            v_sb = big.tile([128, NT, H, 129], BF16)
            xtab_sb = big.tile([128, H, 4, 128], BF16)
            ident_sb = big.tile([128, 128], BF16)
            invq2_sb = big.tile([128, 2, 8], BF16)
            onesr_sb = big.tile([1, 128], BF16)
            eps_sb = big.tile([1, 1], F32)
            nc.vector.memset(eps_sb[:], EPS * DIM)

            # startup-critical DMA order: tiny tables first (invq2 gates the
            # first ssq matmul), then wk/xt pair-slices interleaved in the
            # order the first chunk's terms consume them (hi*hi, hi*lo, lo*hi)
            nc.sync.dma_start(invq2_sb[:], invq2[:])
            nc.sync.dma_start(onesr_sb[:], ones_row[:])
            for p in range(NP4):
                nc.sync.dma_start(w_sb["wk_hi"][:, p, :, :], wps["wk_hi"][:, p, :, :])
                nc.sync.dma_start(xhi_sb[:, p, :, :], xt_hi[:, p, :, :])
            for p in range(NP4):
                nc.sync.dma_start(xlo_sb[:, p, :, :], xt_lo[:, p, :, :])
            for p in range(NP4):
                nc.sync.dma_start(w_sb["wk_lo"][:, p, :, :], wps["wk_lo"][:, p, :, :])
            for nm in ("wq", "wv"):
                for half in ("hi", "lo"):
                    k = f"{nm}_{half}"
                    nc.sync.dma_start(w_sb[k][:], wps[k][:])
            nc.sync.dma_start(xtab_sb[:], xtab[:])
            nc.sync.dma_start(ident_sb[:], ident[:])
            for half in ("hi", "lo"):
                k = f"wo_{half}"
                nc.sync.dma_start(w_sb[k][:], wps[k][:])
            # ones-columns of V (value 32 cancels V's fp8 scale in y/Z)
            for tt in range(NT):
                nc.vector.memset(v_sb[:, tt, :, 128:129], VCOL)

            self_phase1(tc, nc, kt_sb, qt_sb, v_sb, xhi_sb, xlo_sb, w_sb,
                        invq2_sb, onesr_sb, eps_sb)
            self_phase2(tc, nc, kt_sb, qt_sb, v_sb, xtab_sb, ident_sb,
                        w_sb, out)
    nc.compile()
    return nc


def _dr12(nc, ps, whi, wlo, xhi, xlo, mcols, tokcols):
    """12 DoubleRow matmuls accumulating hi*hi + lo*hi + hi*lo into ps."""
    i = 0
    for wt, xt in ((whi, xhi), (whi, xlo), (wlo, xhi)):
        for p in range(NP4):
            nc.tensor.matmul(ps, wt[:, p, :, mcols], xt[:, p, :, tokcols],
                             start=(i == 0), stop=(i == 11), perf_mode=DR)
            i += 1


def self_phase1(tc, nc, kt_sb, qt_sb, v_sb, xhi_sb, xlo_sb, w_sb,
                invq2_sb, onesr_sb, eps_sb):
    with (
        tc.tile_pool(name="scr", bufs=6) as scrp,
        tc.tile_pool(name="rst", bufs=2) as rstp,
        tc.tile_pool(name="pp", bufs=2, space="PSUM") as pp,
        tc.tile_pool(name="sqp", bufs=2, space="PSUM") as sqp,
        tc.tile_pool(name="bcp", bufs=1, space="PSUM") as bcp,
    ):
        pending = []

        def emit_chunk(wnm, dst, c0, cw, tok0, qk):
            """Matmuls + drains + ssq for one chunk; rstd finalize deferred."""
            whi, wlo = w_sb[f"{wnm}_hi"], w_sb[f"{wnm}_lo"]
            tok = slice(tok0 + c0, tok0 + c0 + cw)
            ssq = sqp.tile([1, 512], F32, name="ssq")
            sqs = []
            for m in range(8):
                mc = slice(m * 128, (m + 1) * 128)
                ps = pp.tile([128, 512], F32, tag="ps", name="ps")
                _dr12(nc, ps[:, :cw], whi, wlo, xhi_sb, xlo_sb, mc, tok)
                # drain + square (alternate engines); ssq matmul deferred
                # two m-groups so PE never waits on the drain chain
                sq = scrp.tile([128, 512], BF16, tag="sq", name="sq")
                if m % 2 == 0:
                    nc.scalar.copy(dst[:, m, c0:c0 + cw], ps[:, :cw])
                    nc.vector.tensor_mul(sq[:, :cw], dst[:, m, c0:c0 + cw],
                                         dst[:, m, c0:c0 + cw])
                else:
                    nc.vector.tensor_copy(dst[:, m, c0:c0 + cw], ps[:, :cw])
                    nc.scalar.square(sq[:, :cw], ps[:, :cw])
                sqs.append(sq)
                if m >= 2:
                    nc.tensor.matmul(
                        ssq[:, :cw], invq2_sb[:, qk, m - 2:m - 1],
                        sqs[m - 2][:, :cw], start=(m == 2), stop=False)
            for m in (6, 7):
                nc.tensor.matmul(ssq[:, :cw], invq2_sb[:, qk, m:m + 1],
                                 sqs[m][:, :cw], start=False, stop=(m == 7))
            pending.append((dst, c0, cw, ssq))

        def finalize_chunk():
            """rstd' = 1/(32*sqrt(var+eps)) = 1/sqrt(ssq_true + 1024*eps);
            broadcast and apply. Runs a chunk late so PE never stalls on it."""
            dst, c0, cw, ssq = pending.pop(0)
            sqt = rstp.tile([1, 512], F32, tag="sqt", name="sqt")
            nc.scalar.activation(sqt[:, :cw], ssq[:, :cw], AF.Sqrt,
                                 bias=eps_sb[:], scale=1.0)
            rstd = rstp.tile([1, 512], BF16, tag="rstd", name="rstd")
            with nc.allow_low_precision(reason="bf16 rstd broadcast"):
                nc.vector.reciprocal(rstd[:, :cw], sqt[:, :cw])
            rsb = bcp.tile([128, 512], F32, name="rsb")
            nc.tensor.matmul(rsb[:, :cw], onesr_sb[:], rstd[:, :cw],
                             start=True, stop=True)
            rsb_sb = scrp.tile([128, 512], BF16, tag="rsbsb", name="rsb_sb")
            nc.scalar.copy(rsb_sb[:, :cw], rsb[:, :cw])
            for m in range(8):
                nc.vector.tensor_mul(dst[:, m, c0:c0 + cw],
                                     dst[:, m, c0:c0 + cw],
                                     rsb_sb[:, :cw])

        chunks = ([("wk", kt_sb, c0, cw, 0, 0) for c0, cw in K_CHUNKS]
                  + [("wq", qt_sb, c0, cw, HALO, 1) for c0, cw in Q_CHUNKS])
        for i, (wnm, dst, c0, cw, tok0, qk) in enumerate(chunks):
            emit_chunk(wnm, dst, c0, cw, tok0, qk)
            if i >= 1:
                finalize_chunk()

        # v: natural layout [tok, dim]; ps shaped [128,4,128] so the drain is
        # ONE strided copy into the per-head 129-wide v slots
        whi, wlo = w_sb["wv_hi"], w_sb["wv_lo"]
        for tt in range(NT):
            trange = slice(tt * 128, (tt + 1) * 128)
            for nn in range(2):
                ps = pp.tile([128, 4, 128], F32, tag="psv", name="psv")
                i = 0
                for wt, xt in ((whi, xhi_sb), (whi, xlo_sb), (wlo, xhi_sb)):
                    for p in range(NP4):
                        nc.tensor.matmul(
                            ps[:], xt[:, p, :, trange],
                            wt[:, p, :, nn * 512:(nn + 1) * 512],
                            start=(i == 0), stop=(i == 11), perf_mode=DR)
                        i += 1
                vdst = v_sb[:, tt, nn * 4:(nn + 1) * 4, 0:128]
                if tt % 2 == 0:
                    nc.scalar.copy(vdst, ps[:])
                else:
                    nc.vector.tensor_copy(vdst, ps[:])
            if tt == 1:
                finalize_chunk()  # last wq chunk overlaps the v matmuls
        assert not pending


def self_phase2(tc, nc, kt_sb, qt_sb, v_sb, xtab_sb, ident_sb, w_sb, out):
    with (
        tc.tile_pool(name="exp", bufs=3) as expp,
        tc.tile_pool(name="ysb", bufs=3) as ysbp,
        tc.tile_pool(name="rz", bufs=3) as rzp,
        tc.tile_pool(name="atq", bufs=2) as atqp,
        tc.tile_pool(name="outp", bufs=3) as outp,
        tc.tile_pool(name="sps", bufs=2, space="PSUM") as sps,
        tc.tile_pool(name="ytp", bufs=2, space="PSUM") as ytp,
        tc.tile_pool(name="atp", bufs=2, space="PSUM") as atp,
        tc.tile_pool(name="pso", bufs=2, space="PSUM") as psop,
    ):
        wo_hi, wo_lo = w_sb["wo_hi"], w_sb["wo_lo"]
        for qc in range(4):
            aT_hi = atqp.tile([128, NP4, 2, 256], F8, tag="ahi", name="aT_hi")
            aT_lo = atqp.tile([128, NP4, 2, 256], F8, tag="alo", name="aT_lo")
            for t2 in range(2):
                ti = qc * 2 + t2
                qrange = slice(ti * 128, (ti + 1) * 128)
                xs = 0 if ti == 0 else 2
                for hp in range(4):
                    h0 = 2 * hp
                    # two heads share one PSUM bank / exp / hi / lo op
                    st = sps.tile([128, 2, 2, 128], F32, tag="st", name="st")
                    for i in range(2):
                        for s in range(2):
                            krange = slice((ti + s) * 128, (ti + s + 1) * 128)
                            nc.tensor.matmul(st[:, i, s, :],
                                             kt_sb[:, h0 + i, krange],
                                             qt_sb[:, h0 + i, qrange],
                                             start=True, stop=True)
                    ex_raw = expp.tile([128, 2, 2, 128], BF16, tag="exr",
                                       name="ex_raw")
                    nc.scalar.activation(ex_raw[:], st[:], AF.Exp)
                    ex = expp.tile([128, 2, 2, 128], BF16, tag="ex", name="ex")
                    # mask-multiply is all-SBUF: legal on Pool; split DVE/Pool
                    for i in range(2):
                        eng = nc.vector if hp % 2 == 0 else nc.gpsimd
                        eng.tensor_mul(ex[:, i, :, :], ex_raw[:, i, :, :],
                                       xtab_sb[:, h0 + i, xs:xs + 2, :])
                    y = ytp.tile([128, 2, 129], F32, tag="y", name="y")
                    for i in range(2):
                        for s in range(2):
                            nc.tensor.matmul(y[:, i, :], ex[:, i, s, :],
                                             v_sb[:, ti + s, h0 + i, :],
                                             start=(s == 0), stop=(s == 1))
                    rz2 = rzp.tile([128, 2], F32, tag="rz", name="rz2")
                    nc.vector.reciprocal(rz2[:], y[:, :, 128])
                    y_sb = ysbp.tile([128, 2, 128], BF16, tag="ysb",
                                     name="y_sb")
                    # normalize (reads PSUM, so ACT or DVE only); alternate
                    with nc.allow_low_precision(reason="softmax normalize"):
                        for i in range(2):
                            if hp % 2 == 0:
                                nc.scalar.activation(y_sb[:, i, :],
                                                     y[:, i, 0:128], AF.Copy,
                                                     scale=rz2[:, i:i + 1])
                            else:
                                nc.vector.tensor_scalar_mul(
                                    y_sb[:, i, :], y[:, i, 0:128],
                                    rz2[:, i:i + 1])
                    aT_ps = atp.tile([128, 2, 128], BF16, tag="atps",
                                     name="aT_ps")
                    for i in range(2):
                        nc.tensor.transpose(aT_ps[:, i, :], y_sb[:, i, :],
                                            ident_sb[:])
                    dsl = (slice(None), hp, slice(None),
                           slice(t2 * 128, (t2 + 1) * 128))
                    with nc.allow_low_precision(reason="fp8 hi/lo split"):
                        nc.scalar.copy(aT_hi[dsl], aT_ps[:])
                        nc.vector.tensor_sub(aT_lo[dsl], aT_ps[:], aT_hi[dsl])

            for t2 in range(2):
                for nn in range(2):
                    ps_o = psop.tile([128, 512], F32, name="ps_o")
                    i = 0
                    for at, wt in ((aT_hi, wo_hi), (aT_lo, wo_hi),
                                   (aT_hi, wo_lo)):
                        for p in range(NP4):
                            nc.tensor.matmul(
                                ps_o[:],
                                at[:, p, :, t2 * 128:(t2 + 1) * 128],
                                wt[:, p, :, nn * 512:(nn + 1) * 512],
                                start=(i == 0), stop=(i == 11), perf_mode=DR)
                            i += 1
                    o_sb = outp.tile([128, 512], F32, tag="osb", name="o_sb")
                    if nn == 0:
                        nc.scalar.activation(o_sb[:], ps_o[:], AF.Copy,
                                             scale=1.0 / WS)
                    else:
                        with nc.allow_low_precision(reason="f32 descale"):
                            nc.vector.tensor_scalar_mul(o_sb[:], ps_o[:],
                                                        1.0 / WS)
                    nc.sync.dma_start(
                        out[qc * 256 + t2 * 128: qc * 256 + (t2 + 1) * 128,
                            nn * 512:(nn + 1) * 512],
                        o_sb[:])


def _pair_quant(a, scale):
    hi = (a * scale).astype(E4NP)
    lo = ((a * scale) - hi.astype(np.float32)).astype(E4NP)
    return hi, lo


def _to_pairs(a, ncols):
    """[1024, ncols] -> [128, 4, 2, ncols] (contraction rows on partitions)."""
    return np.ascontiguousarray(
        a.reshape(NP4, 2, 128, ncols).transpose(2, 0, 1, 3))


def _host_constants():
    kj = np.arange(128)[:, None]
    qi = np.arange(128)[None, :]
    slopes = np.asarray(_SLOPES, np.float32)
    xtab = np.zeros((128, H, 4, 128), np.float32)
    for h in range(H):
        relA = kj - 128 - qi                       # previous key tile
        relB = kj - qi                             # same key tile
        xA = np.where((relA <= 0) & (relA >= -WINDOW),
                      np.exp(slopes[h] * relA), 0.0)
        xB = np.where((relB <= 0) & (relB >= -WINDOW),
                      np.exp(slopes[h] * relB), 0.0)
        xtab[:, h, 0, :] = xA                      # first-tile (hf=1 keeps)
        xtab[:, h, 1, :] = xB
        xtab[:, h, 2, :] = xA                      # regular prev-tile
        xtab[:, h, 3, :] = xB
    ident = np.eye(128, dtype=np.float32)
    ones_row = np.ones((1, 128), np.float32)
    return xtab, ident, ones_row


def _make_in_maps(x, wq, wk, wv, wo, q_norm_w, k_norm_w):
    x = np.ascontiguousarray(np.asarray(x, dtype=np.float32))
    wq = np.asarray(wq, dtype=np.float32)
    wk = np.asarray(wk, dtype=np.float32)
    wv = np.asarray(wv, dtype=np.float32)
    wo = np.asarray(wo, dtype=np.float32)
    q_norm_w = np.asarray(q_norm_w, dtype=np.float32)
    k_norm_w = np.asarray(k_norm_w, dtype=np.float32)

    qkw = (q_norm_w * k_norm_w / math.sqrt(HD)).astype(np.float32)
    wk_f = wk * qkw[None, :]

    wpairs = {}
    for nm, w in (("wk", wk_f), ("wq", wq), ("wv", wv), ("wo", wo)):
        hi, lo = _pair_quant(w, WS)
        wpairs[f"{nm}_hi"] = _to_pairs(hi, DIM)
        wpairs[f"{nm}_lo"] = _to_pairs(lo, DIM)

    # ssq contraction columns: 1/(32*qkw)^2 for k, 1/32^2 for q
    invq2 = np.empty((128, 2, 8), np.float32)
    invq2[:, 0, :] = (1.0 / (WS * qkw) ** 2).reshape(8, 128).T
    invq2[:, 1, :] = 1.0 / WS ** 2
    invq2 = invq2.astype(BFNP)

    xtab, ident, ones_row = _host_constants()
    xtab_bf = xtab.astype(BFNP)
    ident_bf = ident.astype(BFNP)
    ones_bf = ones_row.astype(BFNP)

    in_maps = []
    for c in range(8):
        b, hf = c // 2, c % 2
        base = hf * (T // 2)
        xsh = np.zeros((TSH, DIM), dtype=np.float32)
        lo_tok = base - HALO
        if lo_tok < 0:
            xsh[HALO:] = x[b, base: base + QTOK]
        else:
            xsh[:] = x[b, lo_tok: base + QTOK]
        xhi, xlo = _pair_quant(xsh.T, 1.0)
        xt_hi = _to_pairs(xhi, TSH)
        xt_lo = _to_pairs(xlo, TSH)
        xt_c = xtab_bf.copy()
        if hf == 0:
            xt_c[:, :, 0, :] = 0.0                 # halo keys invalid
        im = {"xt_hi": xt_hi, "xt_lo": xt_lo, "xtab": xt_c,
              "ident": ident_bf, "invq2": invq2, "ones_row": ones_bf}
        im.update(wpairs)
        in_maps.append(im)
    return in_maps


def kernel(x, wq, wk, wv, wo, q_norm_w, k_norm_w):
    if "nc" not in _CACHE:
        _CACHE["nc"] = _build_program()
    nc = _CACHE["nc"]
    in_maps = _make_in_maps(x, wq, wk, wv, wo, q_norm_w, k_norm_w)
    _CACHE["in_maps"] = in_maps
    import time as _time
    last_err = None
    for attempt in range(3):
        try:
            res = run_bass_kernel_spmd(nc, in_maps, core_ids=list(range(8)))
            break
        except Exception as e:  # transient NRT/device wedges recover on retry
            last_err = e
            _time.sleep(10 * (attempt + 1))
    else:
        raise last_err

    out = np.empty((B, T, DIM), dtype=np.float32)
    for c in range(8):
        b, hf = c // 2, c % 2
        out[b, hf * QTOK:(hf + 1) * QTOK, :] = res.results[c]["out"]
    return out


# revision 15
# speedup vs baseline: 1.4731x; 1.0701x over previous
"""Trainium2 Bass kernel for nn_CodecAttention (sliding-window ALiBi attention).

Reference computation (B=4, T=2048, DIM=1024, H=8, HD=128, WINDOW=16):
    xq = rms_norm(x @ wq) ; xk = rms_norm(x @ wk) ; xv = x @ wv
    scores = q k^T / sqrt(HD) + alibi_bias  (causal + 16-token sliding window)
    out = softmax(scores) @ v  -> reshape -> @ wo

Sharding: 8 cores = (batch b, sequence half). Each core processes 1024 query
tokens plus a 128-token key/value halo (zeros for the first half), fully
locally -- the attention window (16) never crosses the halo, so no
collectives are needed.

Speed strategy (vs the f32r baseline):
  * All four big projections run as fp8e4 DoubleRow matmuls (0.5 cycles/row,
    K=256 per instruction). Precision is recovered with a hi/lo residual
    split quantized on the HOST: x = x_hi + x_lo, W*32 = W_hi + W_lo, and
    x@W*32 ~= x_hi@W_hi + x_lo@W_hi + x_hi@W_lo (12 DoubleRow instructions
    per K=1024 output tile = 6N cycles vs f32r's 8N).
  * The norm weights (q_norm*k_norm/sqrt(HD)) are folded into wk on the host;
    RMS-norm stays exact because the sum-of-squares matmul contracts with a
    per-partition 1/(32*qkw)^2 column instead of ones. The 32x fp8 scale
    cancels inside rsqrt for q/k, cancels against the "32" ones-column in the
    softmax normalization for v, and is divided out in the output drain.
  * Attention runs in bf16 (full rate at any width): per 128-query tile only
    2 key tiles are computed (down from 3 per 256), the ALiBi bias + causal +
    window mask are ONE bf16 multiply with a precomputed exp(slope*rel) table
    (zero at masked positions), and the softmax denominator comes free as a
    129th "32.0" column appended to V (the 32 cancels V's fp8 scale).
    Normalization is a per-partition ACT copy with scale=1/Z, then a PE
    transpose puts the result back into [head_dim, token] layout for wo.
"""

import math
import os

import numpy as np
import ml_dtypes

os.environ.setdefault("MYCRO_LOCAL_CACHE", "1")

import concourse.mybir as mybir
import concourse.tile as tile
from concourse import bacc
from concourse.bass_utils import run_bass_kernel_spmd

F32 = mybir.dt.float32
BF16 = mybir.dt.bfloat16
F8 = mybir.dt.float8e4
DR = mybir.MatmulPerfMode.DoubleRow
AF = mybir.ActivationFunctionType
ALU = mybir.AluOpType
E4NP = ml_dtypes.float8_e4m3
BFNP = ml_dtypes.bfloat16

B, T, DIM = 4, 2048, 1024
H, HD = 8, 128
WINDOW = 16
EPS = 1e-6
WS = 32.0                  # fp8 weight scale
VCOL = WS                  # value of the ones-column appended to V

HALO = 128                 # key/value halo tokens per shard
TSH = HALO + T // 2        # 1152 tokens per shard
QTOK = T // 2              # 1024 query tokens per shard
NP4 = 4                    # K=1024 -> 4 DoubleRow pairs
NT = TSH // 128            # 9 token tiles
NQT = QTOK // 128          # 8 query tiles
K_CHUNKS = [(0, 384), (384, 384), (768, 384)]        # kT projection chunks
Q_CHUNKS = [(0, 512), (512, 512)]                    # qT projection chunks

_SLOPES = [2.0 ** (-i) for i in range(H)]

_CACHE = {}


def _build_program():
    nc = bacc.Bacc("TRN2", debug=False, target_bir_lowering=False, num_devices=8)

    xt_hi = nc.declare_dram_parameter("xt_hi", [128, NP4, 2, TSH], F8, isOutput=False)
    xt_lo = nc.declare_dram_parameter("xt_lo", [128, NP4, 2, TSH], F8, isOutput=False)
    wps = {}
    for nm in ("wk", "wq", "wv", "wo"):
        for half in ("hi", "lo"):
            wps[f"{nm}_{half}"] = nc.declare_dram_parameter(
                f"{nm}_{half}", [128, NP4, 2, DIM], F8, isOutput=False)
    xtab = nc.declare_dram_parameter("xtab", [128, H, 4, 128], BF16, isOutput=False)
    ident = nc.declare_dram_parameter("ident", [128, 128], BF16, isOutput=False)
    invq2 = nc.declare_dram_parameter("invq2", [128, 2, 8], BF16, isOutput=False)
    ones_row = nc.declare_dram_parameter("ones_row", [1, 128], BF16, isOutput=False)
    out = nc.declare_dram_parameter("out", [QTOK, DIM], F32, isOutput=True)

    with tile.TileContext(nc) as tc:
        with tc.tile_pool(name="big", bufs=1) as big:
            # ---- persistent SBUF tensors ----
            xhi_sb = big.tile([128, NP4, 2, TSH], F8)
            xlo_sb = big.tile([128, NP4, 2, TSH], F8)
            w_sb = {k: big.tile([128, NP4, 2, DIM], F8, name=f"w_{k}")
                    for k in wps}
            kt_sb = big.tile([128, H, TSH], BF16)
            qt_sb = big.tile([128, H, QTOK], BF16)
            v_sb = big.tile([128, NT, H, 129], BF16)
            xtab_sb = big.tile([128, H, 4, 128], BF16)
            ident_sb = big.tile([128, 128], BF16)
            invq2_sb = big.tile([128, 2, 8], BF16)
            onesr_sb = big.tile([1, 128], BF16)
            eps_sb = big.tile([1, 1], F32)
            nc.vector.memset(eps_sb[:], EPS * DIM)

            # startup-critical DMA order: tiny tables first (invq2 gates the
            # first ssq matmul), then wk/xt pair-slices interleaved in the
            # order the first chunk's terms consume them (hi*hi, hi*lo, lo*hi)
            # chunk 0 (tokens 0:384) only needs the leading xt columns; DMA
            # those first so the first m-groups start ~5us earlier. Each
            # dma_start costs ~565ns of SP issue time, so the first wave is
            # exactly the 8 transfers the first m-group consumes.
            C0 = K_CHUNKS[0][1]
            for p in range(NP4):
                nc.sync.dma_start(w_sb["wk_hi"][:, p, :, :], wps["wk_hi"][:, p, :, :])
                nc.sync.dma_start(xhi_sb[:, p, :, 0:C0], xt_hi[:, p, :, 0:C0])
            nc.sync.dma_start(invq2_sb[:], invq2[:])
            nc.sync.dma_start(onesr_sb[:], ones_row[:])
            for p in range(NP4):
                nc.sync.dma_start(xlo_sb[:, p, :, 0:C0], xt_lo[:, p, :, 0:C0])
            for p in range(NP4):
                nc.sync.dma_start(w_sb["wk_lo"][:, p, :, :], wps["wk_lo"][:, p, :, :])
            C1 = 768
            for p in range(NP4):
                nc.sync.dma_start(xhi_sb[:, p, :, C0:C1], xt_hi[:, p, :, C0:C1])
                nc.sync.dma_start(xlo_sb[:, p, :, C0:C1], xt_lo[:, p, :, C0:C1])
            for p in range(NP4):
                nc.sync.dma_start(xhi_sb[:, p, :, C1:TSH], xt_hi[:, p, :, C1:TSH])
                nc.sync.dma_start(xlo_sb[:, p, :, C1:TSH], xt_lo[:, p, :, C1:TSH])
            for nm in ("wq", "wv"):
                for half in ("hi", "lo"):
                    k = f"{nm}_{half}"
                    nc.sync.dma_start(w_sb[k][:], wps[k][:])
            nc.sync.dma_start(xtab_sb[:], xtab[:])
            nc.sync.dma_start(ident_sb[:], ident[:])
            for half in ("hi", "lo"):
                k = f"wo_{half}"
                nc.sync.dma_start(w_sb[k][:], wps[k][:])
            # ones-columns of V (value 32 cancels V's fp8 scale in y/Z)
            for tt in range(NT):
                nc.vector.memset(v_sb[:, tt, :, 128:129], VCOL)

            self_phase1(tc, nc, kt_sb, qt_sb, v_sb, xhi_sb, xlo_sb, w_sb,
                        invq2_sb, onesr_sb, eps_sb)
            self_phase2(tc, nc, kt_sb, qt_sb, v_sb, xhi_sb, xlo_sb,
                        xtab_sb, ident_sb, w_sb, out)
    nc.compile()
    return nc


def _dr12(nc, ps, whi, wlo, xhi, xlo, mcols, tokcols):
    """12 DoubleRow matmuls accumulating hi*hi + lo*hi + hi*lo into ps."""
    i = 0
    for wt, xt in ((whi, xhi), (whi, xlo), (wlo, xhi)):
        for p in range(NP4):
            nc.tensor.matmul(ps, wt[:, p, :, mcols], xt[:, p, :, tokcols],
                             start=(i == 0), stop=(i == 11), perf_mode=DR)
            i += 1


def self_phase1(tc, nc, kt_sb, qt_sb, v_sb, xhi_sb, xlo_sb, w_sb,
                invq2_sb, onesr_sb, eps_sb):
    with (
        tc.tile_pool(name="scr", bufs=6) as scrp,
        tc.tile_pool(name="rst", bufs=2) as rstp,
        tc.tile_pool(name="pp", bufs=2, space="PSUM") as pp,
        tc.tile_pool(name="sqp", bufs=2, space="PSUM") as sqp,
        tc.tile_pool(name="bcp", bufs=1, space="PSUM") as bcp,
    ):
        pending = []

        def emit_chunk(wnm, dst, c0, cw, tok0, qk):
            """Matmuls + drains + ssq for one chunk; rstd finalize deferred."""
            whi, wlo = w_sb[f"{wnm}_hi"], w_sb[f"{wnm}_lo"]
            tok = slice(tok0 + c0, tok0 + c0 + cw)
            ssq = sqp.tile([1, 512], F32, name="ssq")
            sqs = []
            for m in range(8):
                mc = slice(m * 128, (m + 1) * 128)
                ps = pp.tile([128, 512], F32, tag="ps", name="ps")
                _dr12(nc, ps[:, :cw], whi, wlo, xhi_sb, xlo_sb, mc, tok)
                # drain + square (alternate engines); ssq matmul deferred
                # two m-groups so PE never waits on the drain chain
                sq = scrp.tile([128, 512], BF16, tag="sq", name="sq")
                if m % 2 == 0:
                    nc.scalar.copy(dst[:, m, c0:c0 + cw], ps[:, :cw])
                    nc.vector.tensor_mul(sq[:, :cw], dst[:, m, c0:c0 + cw],
                                         dst[:, m, c0:c0 + cw])
                else:
                    nc.vector.tensor_copy(dst[:, m, c0:c0 + cw], ps[:, :cw])
                    nc.scalar.square(sq[:, :cw], ps[:, :cw])
                sqs.append(sq)
                if m >= 2:
                    nc.tensor.matmul(
                        ssq[:, :cw], invq2_sb[:, qk, m - 2:m - 1],
                        sqs[m - 2][:, :cw], start=(m == 2), stop=False)
            for m in (6, 7):
                nc.tensor.matmul(ssq[:, :cw], invq2_sb[:, qk, m:m + 1],
                                 sqs[m][:, :cw], start=False, stop=(m == 7))
            pending.append((dst, c0, cw, ssq))

        def finalize_chunk():
            """rstd' = 1/(32*sqrt(var+eps)) = 1/sqrt(ssq_true + 1024*eps);
            broadcast and apply. Runs a chunk late so PE never stalls on it."""
            dst, c0, cw, ssq = pending.pop(0)
            sqt = rstp.tile([1, 512], F32, tag="sqt", name="sqt")
            nc.scalar.activation(sqt[:, :cw], ssq[:, :cw], AF.Sqrt,
                                 bias=eps_sb[:], scale=1.0)
            rstd = rstp.tile([1, 512], BF16, tag="rstd", name="rstd")
            with nc.allow_low_precision(reason="bf16 rstd broadcast"):
                nc.vector.reciprocal(rstd[:, :cw], sqt[:, :cw])
            rsb = bcp.tile([128, 512], F32, name="rsb")
            nc.tensor.matmul(rsb[:, :cw], onesr_sb[:], rstd[:, :cw],
                             start=True, stop=True)
            rsb_sb = scrp.tile([128, 512], BF16, tag="rsbsb", name="rsb_sb")
            nc.scalar.copy(rsb_sb[:, :cw], rsb[:, :cw])
            for m in range(8):
                nc.vector.tensor_mul(dst[:, m, c0:c0 + cw],
                                     dst[:, m, c0:c0 + cw],
                                     rsb_sb[:, :cw])

        chunks = ([("wk", kt_sb, c0, cw, 0, 0) for c0, cw in K_CHUNKS]
                  + [("wq", qt_sb, c0, cw, HALO, 1) for c0, cw in Q_CHUNKS])
        for i, (wnm, dst, c0, cw, tok0, qk) in enumerate(chunks):
            emit_chunk(wnm, dst, c0, cw, tok0, qk)
            if i >= 1:
                finalize_chunk()
        finalize_chunk()
        assert not pending


def self_phase2(tc, nc, kt_sb, qt_sb, v_sb, xhi_sb, xlo_sb, xtab_sb, ident_sb,
                w_sb, out):
    with (
        tc.tile_pool(name="exp", bufs=4) as expp,
        tc.tile_pool(name="ysb", bufs=4) as ysbp,
        tc.tile_pool(name="rz", bufs=4) as rzp,
        tc.tile_pool(name="atq", bufs=2) as atqp,
        tc.tile_pool(name="outp", bufs=3) as outp,
        tc.tile_pool(name="sps", bufs=2, space="PSUM") as sps,
        tc.tile_pool(name="ytp", bufs=2, space="PSUM") as ytp,
        tc.tile_pool(name="atp", bufs=2, space="PSUM") as atp,
        tc.tile_pool(name="pso", bufs=2, space="PSUM") as psop,
    ):
        wo_hi, wo_lo = w_sb["wo_hi"], w_sb["wo_lo"]
        vhi, vlo = w_sb["wv_hi"], w_sb["wv_lo"]
        aT = {}

        def emit_v(tt):
            trange = slice(tt * 128, (tt + 1) * 128)
            for nn in range(2):
                ps = psop.tile([128, 512], F32, tag="pso", name="ps_v")
                i = 0
                for wt, xt in ((vhi, xhi_sb), (vhi, xlo_sb), (vlo, xhi_sb)):
                    for p in range(NP4):
                        nc.tensor.matmul(
                            ps[:], xt[:, p, :, trange],
                            wt[:, p, :, nn * 512:(nn + 1) * 512],
                            start=(i == 0), stop=(i == 11), perf_mode=DR)
                        i += 1
                vdst = v_sb[:, tt, nn * 4:(nn + 1) * 4, 0:128]
                if tt % 2 == 0:
                    nc.scalar.copy(vdst, ps[:])
                else:
                    nc.vector.tensor_copy(vdst, ps[:])

        def emit_attention(qc):
            aT_hi = atqp.tile([128, NP4, 2, 256], F8, tag="ahi", name="aT_hi")
            aT_lo = atqp.tile([128, NP4, 2, 256], F8, tag="alo", name="aT_lo")
            aT[qc] = (aT_hi, aT_lo)
            for t2 in range(2):
                ti = qc * 2 + t2
                qrange = slice(ti * 128, (ti + 1) * 128)
                xs = 0 if ti == 0 else 2
                for hp in range(4):
                    h0 = 2 * hp
                    # two heads share one PSUM bank / exp / hi / lo op
                    st = sps.tile([128, 2, 2, 128], F32, tag="st", name="st")
                    for i in range(2):
                        for s in range(2):
                            krange = slice((ti + s) * 128, (ti + s + 1) * 128)
                            nc.tensor.matmul(st[:, i, s, :],
                                             kt_sb[:, h0 + i, krange],
                                             qt_sb[:, h0 + i, qrange],
                                             start=True, stop=True)
                    ex_raw = expp.tile([128, 2, 2, 128], BF16, tag="exr",
                                       name="ex_raw")
                    nc.scalar.activation(ex_raw[:], st[:], AF.Exp)
                    ex = expp.tile([128, 2, 2, 128], BF16, tag="ex", name="ex")
                    # mask-multiply is all-SBUF: legal on Pool; split DVE/Pool
                    for i in range(2):
                        eng = nc.vector if i == 0 else nc.gpsimd
                        eng.tensor_mul(ex[:, i, :, :], ex_raw[:, i, :, :],
                                       xtab_sb[:, h0 + i, xs:xs + 2, :])
                    y = ytp.tile([128, 2, 129], F32, tag="y", name="y")
                    for i in range(2):
                        for s in range(2):
                            nc.tensor.matmul(y[:, i, :], ex[:, i, s, :],
                                             v_sb[:, ti + s, h0 + i, :],
                                             start=(s == 0), stop=(s == 1))
                    rz2 = rzp.tile([128, 2], F32, tag="rz", name="rz2")
                    nc.vector.reciprocal(rz2[:], y[:, :, 128])
                    y_sb = ysbp.tile([128, 2, 128], BF16, tag="ysb",
                                     name="y_sb")
                    # normalize on ACT (per-partition scale); PSUM-legal
                    with nc.allow_low_precision(reason="softmax normalize"):
                        for i in range(2):
                            nc.scalar.activation(y_sb[:, i, :],
                                                 y[:, i, 0:128], AF.Copy,
                                                 scale=rz2[:, i:i + 1])
                    aT_ps = atp.tile([128, 2, 128], BF16, tag="atps",
                                     name="aT_ps")
                    for i in range(2):
                        nc.tensor.transpose(aT_ps[:, i, :], y_sb[:, i, :],
                                            ident_sb[:])
                    dsl = (slice(None), hp, slice(None),
                           slice(t2 * 128, (t2 + 1) * 128))
                    with nc.allow_low_precision(reason="fp8 hi/lo split"):
                        nc.vector.tensor_copy(aT_hi[dsl], aT_ps[:])
                        nc.vector.tensor_sub(aT_lo[dsl], aT_ps[:], aT_hi[dsl])

        def emit_wo(qc):
            aT_hi, aT_lo = aT.pop(qc)
            for t2 in range(2):
                for nn in range(2):
                    ps_o = psop.tile([128, 512], F32, tag="pso", name="ps_o")
                    i = 0
                    for at, wt in ((aT_hi, wo_hi), (aT_lo, wo_hi),
                                   (aT_hi, wo_lo)):
                        for p in range(NP4):
                            nc.tensor.matmul(
                                ps_o[:],
                                at[:, p, :, t2 * 128:(t2 + 1) * 128],
                                wt[:, p, :, nn * 512:(nn + 1) * 512],
                                start=(i == 0), stop=(i == 11), perf_mode=DR)
                            i += 1
                    o_sb = outp.tile([128, 512], F32, tag="osb", name="o_sb")
                    if nn == 0:
                        nc.scalar.activation(o_sb[:], ps_o[:], AF.Copy,
                                             scale=1.0 / WS)
                    else:
                        with nc.allow_low_precision(reason="f32 descale"):
                            nc.vector.tensor_scalar_mul(o_sb[:], ps_o[:],
                                                        1.0 / WS)
                    nc.sync.dma_start(
                        out[qc * 256 + t2 * 128: qc * 256 + (t2 + 1) * 128,
                            nn * 512:(nn + 1) * 512],
                        o_sb[:])

        # v tiles, attention blocks and wo interleaved so PE never drains:
        # A(qc) needs v tiles <= 2qc+2; wo(qc) delayed one block
        emit_v(0); emit_v(1); emit_v(2)
        emit_attention(0)
        emit_v(3); emit_v(4)
        emit_attention(1); emit_wo(0)
        emit_v(5); emit_v(6)
        emit_attention(2); emit_wo(1)
        emit_v(7); emit_v(8)
        emit_attention(3); emit_wo(2)
        emit_wo(3)


def _pair_quant(a, scale):
    hi = (a * scale).astype(E4NP)
    lo = ((a * scale) - hi.astype(np.float32)).astype(E4NP)
    return hi, lo


def _to_pairs(a, ncols):
    """[1024, ncols] -> [128, 4, 2, ncols] (contraction rows on partitions)."""
    return np.ascontiguousarray(
        a.reshape(NP4, 2, 128, ncols).transpose(2, 0, 1, 3))


def _host_constants():
    kj = np.arange(128)[:, None]
    qi = np.arange(128)[None, :]
    slopes = np.asarray(_SLOPES, np.float32)
    xtab = np.zeros((128, H, 4, 128), np.float32)
    for h in range(H):
        relA = kj - 128 - qi                       # previous key tile
        relB = kj - qi                             # same key tile
        xA = np.where((relA <= 0) & (relA >= -WINDOW),
                      np.exp(slopes[h] * relA), 0.0)
        xB = np.where((relB <= 0) & (relB >= -WINDOW),
                      np.exp(slopes[h] * relB), 0.0)
        xtab[:, h, 0, :] = xA                      # first-tile (hf=1 keeps)
        xtab[:, h, 1, :] = xB
        xtab[:, h, 2, :] = xA                      # regular prev-tile
        xtab[:, h, 3, :] = xB
    ident = np.eye(128, dtype=np.float32)
    ones_row = np.ones((1, 128), np.float32)
    return xtab, ident, ones_row


def _make_in_maps(x, wq, wk, wv, wo, q_norm_w, k_norm_w):
    x = np.ascontiguousarray(np.asarray(x, dtype=np.float32))
    wq = np.asarray(wq, dtype=np.float32)
    wk = np.asarray(wk, dtype=np.float32)
    wv = np.asarray(wv, dtype=np.float32)
    wo = np.asarray(wo, dtype=np.float32)
    q_norm_w = np.asarray(q_norm_w, dtype=np.float32)
    k_norm_w = np.asarray(k_norm_w, dtype=np.float32)

    qkw = (q_norm_w * k_norm_w / math.sqrt(HD)).astype(np.float32)
    wk_f = wk * qkw[None, :]

    wpairs = {}
    for nm, w in (("wk", wk_f), ("wq", wq), ("wv", wv), ("wo", wo)):
        hi, lo = _pair_quant(w, WS)
        wpairs[f"{nm}_hi"] = _to_pairs(hi, DIM)
        wpairs[f"{nm}_lo"] = _to_pairs(lo, DIM)

    # ssq contraction columns: 1/(32*qkw)^2 for k, 1/32^2 for q
    invq2 = np.empty((128, 2, 8), np.float32)
    invq2[:, 0, :] = (1.0 / (WS * qkw) ** 2).reshape(8, 128).T
    invq2[:, 1, :] = 1.0 / WS ** 2
    invq2 = invq2.astype(BFNP)

    xtab, ident, ones_row = _host_constants()
    xtab_bf = xtab.astype(BFNP)
    ident_bf = ident.astype(BFNP)
    ones_bf = ones_row.astype(BFNP)

    in_maps = []
    for c in range(8):
        b, hf = c // 2, c % 2
        base = hf * (T // 2)
        xsh = np.zeros((TSH, DIM), dtype=np.float32)
        lo_tok = base - HALO
        if lo_tok < 0:
            xsh[HALO:] = x[b, base: base + QTOK]
        else:
            xsh[:] = x[b, lo_tok: base + QTOK]
        xhi, xlo = _pair_quant(xsh.T, 1.0)
        xt_hi = _to_pairs(xhi, TSH)
        xt_lo = _to_pairs(xlo, TSH)
        xt_c = xtab_bf.copy()
        if hf == 0:
            xt_c[:, :, 0, :] = 0.0                 # halo keys invalid
        im = {"xt_hi": xt_hi, "xt_lo": xt_lo, "xtab": xt_c,
              "ident": ident_bf, "invq2": invq2, "ones_row": ones_bf}
        im.update(wpairs)
        in_maps.append(im)
    return in_maps


def kernel(x, wq, wk, wv, wo, q_norm_w, k_norm_w):
    if "nc" not in _CACHE:
        _CACHE["nc"] = _build_program()
    nc = _CACHE["nc"]
    in_maps = _make_in_maps(x, wq, wk, wv, wo, q_norm_w, k_norm_w)
    _CACHE["in_maps"] = in_maps
    import time as _time
    last_err = None
    for attempt in range(3):
        try:
            res = run_bass_kernel_spmd(nc, in_maps, core_ids=list(range(8)))
            break
        except Exception as e:  # transient NRT/device wedges recover on retry
            last_err = e
            _time.sleep(10 * (attempt + 1))
    else:
        raise last_err

    out = np.empty((B, T, DIM), dtype=np.float32)
    for c in range(8):
        b, hf = c // 2, c % 2
        out[b, hf * QTOK:(hf + 1) * QTOK, :] = res.results[c]["out"]
    return out
